# revision 7
# baseline (speedup 1.0000x reference)
"""Trainium2 Bass kernel for nn_BiRNNModel_51771535786398.

Math (per token, h=0 GRU cell applied pointwise, fwd+bwd weights, L=2):
  gi = x @ W_ih[l].T + b_ih[l]          (3H gates: r | z | n)
  r  = sigmoid(gi_r + bhr)
  z  = sigmoid(gi_z + bhz)
  n  = tanh(gi_n + r * bhn)
  out = (1 - z) * n
Forward outputs go to rows s*L+l, "backward" outputs (same math, bwd
weights, token permutation idx[s] = (-s) % S) go to rows S*L + idx(s)*L+l.
Because there is no cross-timestep dependence, we compute bwd outputs from
the *unpermuted* tokens and write them to permuted rows (idx is an
involution), realized as negative-stride store DMAs.

Sharding: pure data parallel over batch (B=32 -> 4 per core, 8 cores).

Device layout choice: tokens on partitions (PSUM partition dim = token),
gate columns on the free dim. Gate column layout (3072 wide):
  [ R: 1024 | Z: 1024 | N: 1024 ], each block = (fwd-l0, fwd-l1, bwd-l0,
  bwd-l1) x 256 h.  Z-block weights and biases are NEGATED so that a single
  merged sigmoid over [R|Z] yields r and z' = 1-z directly.
"""

import os
import sys

sys.path.insert(0, "/opt/trn_rl_repo")

import numpy as np
import ml_dtypes

B, S, I, H, L = 32, 4096, 256, 256, 2
NCORES = 8
BPC = B // NCORES          # batch rows per core
NT = 128                   # tokens per tile
SB_PER_B = S // NT         # 32 token-tiles per batch row
NTILES = BPC * SB_PER_B    # 128 tiles per core
GCOLS = 3072               # gate columns (R|Z|N x 4 (dir,l) x 256 h)

BF16 = ml_dtypes.bfloat16

_CACHE = {}


def _prep_weights(W_ih_fwd, b_ih_fwd, b_hh_fwd, W_ih_bwd, b_ih_bwd, b_hh_bwd):
    """Build rhs weight tiles / bias tiles in the device gate-column layout.

    Returns (w_np [2,128,3072] bf16, bias_np [128,3072] f32,
             bhn_np [128,1024] bf16).
    """
    Wd = [W_ih_fwd, W_ih_fwd, W_ih_bwd, W_ih_bwd]
    bid = [b_ih_fwd, b_ih_fwd, b_ih_bwd, b_ih_bwd]
    bhd = [b_hh_fwd, b_hh_fwd, b_hh_bwd, b_hh_bwd]

    w = np.zeros((2, 128, GCOLS), np.float32)
    bias = np.zeros(GCOLS, np.float32)
    bhn = np.zeros(1024, np.float32)
    for dl in range(4):
        l = dl % 2
        Wl = np.asarray(Wd[dl][l], np.float32)      # (3H, I)
        bil = np.asarray(bid[dl][l], np.float32)    # (3H,)
        bhl = np.asarray(bhd[dl][l], np.float32)
        sl = slice(dl * 256, (dl + 1) * 256)
        for k in range(2):
            isel = slice(k * 128, (k + 1) * 128)
            # R block: cols [0:1024)
            w[k, :, 0:1024][:, sl] = Wl[0:H, isel].T
            # Z block negated: cols [1024:2048)
            w[k, :, 1024:2048][:, sl] = -Wl[H : 2 * H, isel].T
            # N block: cols [2048:3072)
            w[k, :, 2048:3072][:, sl] = Wl[2 * H : 3 * H, isel].T
        bias[0:1024][sl] = bil[0:H] + bhl[0:H]
        bias[1024:2048][sl] = -(bil[H : 2 * H] + bhl[H : 2 * H])
        bias[2048:3072][sl] = bil[2 * H : 3 * H]
        bhn[sl] = bhl[2 * H : 3 * H]

    w_np = w.astype(BF16)
    bias_np = np.ascontiguousarray(np.broadcast_to(bias, (128, GCOLS)), np.float32)
    bhn_np = np.ascontiguousarray(np.broadcast_to(bhn, (128, 1024))).astype(BF16)
    return w_np, bias_np, bhn_np


def _build_nc():
    import concourse.bass as bass
    import concourse.mybir as mybir
    from concourse import bacc
    import concourse.tile as tile
    from concourse.alu_op_type import AluOpType

    AF = mybir.ActivationFunctionType
    f32 = mybir.dt.float32
    bf16 = mybir.dt.bfloat16

    nc = bacc.Bacc(
        "TRN2", target_bir_lowering=False, debug=False, num_devices=NCORES
    )
    x_in = nc.dram_tensor("x", [BPC, S, I], f32, kind="ExternalInput").ap()
    w_in = nc.dram_tensor("w", [2, 128, GCOLS], bf16, kind="ExternalInput").ap()
    bias_in = nc.dram_tensor("bias", [128, GCOLS], f32, kind="ExternalInput").ap()
    bhn_in = nc.dram_tensor("bhn", [128, 1024], bf16, kind="ExternalInput").ap()
    out_t = nc.dram_tensor("out", [BPC, 2 * S * L, H], f32, kind="ExternalOutput")

    OUT_B = 2 * S * L * H       # flat elems per batch row
    BWD_OFF = S * L * H         # flat offset of the bwd half within a batch row

    with tile.TileContext(nc) as tc:
        with (
            tc.tile_pool(name="const", bufs=1) as cpool,
            tc.tile_pool(name="xload", bufs=3) as xpool,
            tc.tile_pool(name="xt", bufs=4) as xtpool,
            tc.tile_pool(name="work", bufs=3) as wpool,
            tc.tile_pool(name="outp", bufs=4) as opool,
            tc.tile_pool(name="ps_rz", bufs=1, space="PSUM") as prz,
            tc.tile_pool(name="ps_n", bufs=2, space="PSUM") as pnp,
        ):
            w0 = cpool.tile([128, GCOLS], bf16, name="w0")
            w1 = cpool.tile([128, GCOLS], bf16, name="w1")
            bias_sb = cpool.tile([128, GCOLS], f32, name="bias_sb")
            bhn_sb = cpool.tile([128, 1024], bf16, name="bhn_sb")
            nc.sync.dma_start(out=w0[:], in_=w_in[0])
            nc.sync.dma_start(out=w1[:], in_=w_in[1])
            nc.sync.dma_start(out=bias_sb[:], in_=bias_in)
            nc.sync.dma_start(out=bhn_sb[:], in_=bhn_in)
            wk = [w0, w1]

            for it4 in range(NTILES // 4):
                b = (it4 * 4) // SB_PER_B
                sb4 = (it4 * 4) % SB_PER_B
                xin4 = xpool.tile([128, 4 * I], bf16, name="xin4")
                src = x_in[b, sb4 * NT : (sb4 + 4) * NT, :].rearrange(
                    "(j p) i -> p j i", p=128
                )
                # SWDGE cast DMA: f32 DRAM -> bf16 SBUF
                nc.gpsimd.dma_start(out=xin4[:], in_=src)

                for j in range(4):
                    t0 = (sb4 + j) * NT
                    xT = xtpool.tile([128, 2 * NT], bf16, name="xT")
                    for k in range(2):
                        nc.sync.dma_start_transpose(
                            out=xT[:, k * NT : (k + 1) * NT],
                            in_=xin4[:, j * I + k * 128 : j * I + (k + 1) * 128],
                        )

                    ps_n = pnp.tile([128, 1024], f32, name="ps_n")
                    ps_rz = prz.tile([128, 2048], f32, name="ps_rz")
                    # Gate-column 512-blocks alternate fwd/bwd:
                    #   rz: [r-fwd, r-bwd, z-fwd, z-bwd], n: [n-fwd, n-bwd].
                    # Bwd blocks use the column-REVERSED stationary xT so psum
                    # partition p holds token t0+127-p; the elementwise chain
                    # is pointwise so this stays consistent, and the bwd store
                    # becomes an ascending-stride DMA.
                    # column-reversed copy of xT (per k-chunk) for bwd blocks;
                    # matmul weight APs reject negative strides, so materialize
                    # via a DVE copy (step -1 input is a supported fast path).
                    xTr = xtpool.tile([128, 2 * NT], bf16, name="xTr")
                    for k in range(2):
                        rev_view = bass.AP(
                            xT.tensor,
                            xT.offset + (k + 1) * NT - 1,
                            [list(xT.ap[0]), [-1, NT]],
                        )
                        nc.vector.tensor_copy(xTr[:, k * NT : (k + 1) * NT], rev_view)
                    for k in range(2):
                        xk = xT[:, k * NT : (k + 1) * NT]
                        xkr = xTr[:, k * NT : (k + 1) * NT]
                        for rev in (0, 1):
                            lhsT = xkr if rev else xk
                            for gt in range(3):  # r, z, n blocks
                                col = gt * 1024 + rev * 512
                                dst = (
                                    ps_n[:, rev * 512 : (rev + 1) * 512]
                                    if gt == 2
                                    else ps_rz[:, (gt * 2 + rev) * 512 : (gt * 2 + rev + 1) * 512]
                                )
                                nc.tensor.matmul(
                                    dst,
                                    lhsT,
                                    wk[k][:, col : col + 512],
                                    start=(k == 0),
                                    stop=(k == 1),
                                )

                    nb_sb = wpool.tile([128, 1024], bf16, name="nb_sb")
                    nc.vector.tensor_tensor(
                        nb_sb[:], ps_n[:], bias_sb[:, 2048:GCOLS], AluOpType.add
                    )
                    rz_pre = wpool.tile([128, 2048], bf16, name="rz_pre")
                    nc.vector.tensor_tensor(
                        rz_pre[:], ps_rz[:], bias_sb[:, 0:2048], AluOpType.add
                    )
                    rz_act = wpool.tile([128, 2048], bf16, name="rz_act")
                    nc.scalar.activation(rz_act[:], rz_pre[:], AF.Sigmoid)
                    tmul = wpool.tile([128, 1024], bf16, name="tmul")
                    nc.vector.tensor_tensor(
                        tmul[:], rz_act[:, 0:1024], bhn_sb[:], AluOpType.mult
                    )
                    pre_n = wpool.tile([128, 1024], bf16, name="pre_n")
                    nc.vector.tensor_tensor(
                        pre_n[:], nb_sb[:], tmul[:], AluOpType.add
                    )
                    n_sb = wpool.tile([128, 1024], bf16, name="n_sb")
                    nc.scalar.activation(n_sb[:], pre_n[:], AF.Tanh)
                    out_sb = opool.tile([128, 1024], f32, name="out_sb")
                    nc.gpsimd.tensor_tensor(
                        out_sb[:], rz_act[:, 1024:2048], n_sb[:], AluOpType.mult
                    )

                    base = b * OUT_B
                    fwd = bass.AP(out_t, base + t0 * 512, [[512, 128], [1, 512]])
                    nc.sync.dma_start(out=fwd, in_=out_sb[:, 0:512])
                    # bwd partitions hold tokens reversed (p <-> t0+127-p), so
                    # dest rows q = S - t0 - 127 + p ascend with p.
                    bbase = base + BWD_OFF
                    if t0 == 0:
                        # p=0..126 -> q=3969..4095 ; p=127 (token 0) -> q=0
                        rest = bass.AP(
                            out_t, bbase + (S - 127) * 512, [[512, 127], [1, 512]]
                        )
                        nc.sync.dma_start(out=rest, in_=out_sb[0:127, 512:1024])
                        one = bass.AP(out_t, bbase, [[512, 1], [1, 512]])
                        nc.sync.dma_start(out=one, in_=out_sb[127:128, 512:1024])
                    else:
                        bwd = bass.AP(
                            out_t,
                            bbase + (S - t0 - 127) * 512,
                            [[512, 128], [1, 512]],
                        )
                        nc.sync.dma_start(out=bwd, in_=out_sb[:, 512:1024])

    nc.compile()
    return nc


def _get_nc():
    if "nc" not in _CACHE:
        _CACHE["nc"] = _build_nc()
    return _CACHE["nc"]


def kernel(
    input,
    W_ih_fwd,
    W_hh_fwd,
    b_ih_fwd,
    b_hh_fwd,
    W_ih_bwd,
    W_hh_bwd,
    b_ih_bwd,
    b_hh_bwd,
    _trace=False,
):
    from concourse.bass_utils import run_bass_kernel_spmd

    x = np.asarray(input, np.float32)
    w_np, bias_np, bhn_np = _prep_weights(
        np.asarray(W_ih_fwd, np.float32),
        np.asarray(b_ih_fwd, np.float32),
        np.asarray(b_hh_fwd, np.float32),
        np.asarray(W_ih_bwd, np.float32),
        np.asarray(b_ih_bwd, np.float32),
        np.asarray(b_hh_bwd, np.float32),
    )

    nc = _get_nc()
    in_maps = []
    for c in range(NCORES):
        in_maps.append(
            {
                "x": np.ascontiguousarray(x[c * BPC : (c + 1) * BPC]),
                "w": w_np,
                "bias": bias_np,
                "bhn": bhn_np,
            }
        )
    res = run_bass_kernel_spmd(
        nc, in_maps, core_ids=list(range(NCORES)), trace=_trace
    )
    out = np.concatenate([r["out"] for r in res.results], axis=0)
    if _trace:
        _CACHE["last_results"] = res
    return out


# revision 25
# speedup vs baseline: 81.4198x; 81.4198x over previous
"""Trainium2 Bass kernel for nn_BiRNNModel_51771535786398.

Math (per token, h=0 GRU cell applied pointwise, fwd+bwd weights, L=2):
  gi = x @ W_ih[l].T + b_ih[l]          (3H gates: r | z | n)
  r  = sigmoid(gi_r + bhr)
  z  = sigmoid(gi_z + bhz)
  n  = tanh(gi_n + r * bhn)
  out = (1 - z) * n
Forward outputs go to rows s*L+l, "backward" outputs (same math, bwd
weights, token permutation idx[s] = (-s) % S) go to rows S*L + idx(s)*L+l.
Because there is no cross-timestep dependence, we compute bwd outputs from
the *unpermuted* tokens and write them to permuted rows (idx is an
involution), realized as negative-stride store DMAs.

Sharding: pure data parallel over batch (B=32 -> 4 per core, 8 cores).

Device layout choice: tokens on partitions (PSUM partition dim = token),
gate columns on the free dim. Gate column layout (3072 wide):
  [ R: 1024 | Z: 1024 | N: 1024 ], each block = (fwd-l0, fwd-l1, bwd-l0,
  bwd-l1) x 256 h.  Z-block weights and biases are NEGATED so that a single
  merged sigmoid over [R|Z] yields r and z' = 1-z directly.
"""

import os
import sys

sys.path.insert(0, "/opt/trn_rl_repo")

import numpy as np
import ml_dtypes

B, S, I, H, L = 32, 4096, 256, 256, 2
NCORES = 8
BPC = B // NCORES          # batch rows per core
NT = 128                   # tokens per tile
SB_PER_B = S // NT         # 32 token-tiles per batch row
NTILES = BPC * SB_PER_B    # 128 tiles per core
GCOLS = 3072               # gate columns (R|Z|N x 4 (dir,l) x 256 h)

BF16 = ml_dtypes.bfloat16

_CACHE = {}


def _prep_weights(W_ih_fwd, b_ih_fwd, b_hh_fwd, W_ih_bwd, b_ih_bwd, b_hh_bwd):
    """Build rhs weight tiles / bias tiles in the device gate-column layout.

    Returns (w_np [2,128,3072] bf16, bias_np [128,3072] f32,
             bhn_np [128,1024] bf16).
    """
    Wd = [W_ih_fwd, W_ih_fwd, W_ih_bwd, W_ih_bwd]
    bid = [b_ih_fwd, b_ih_fwd, b_ih_bwd, b_ih_bwd]
    bhd = [b_hh_fwd, b_hh_fwd, b_hh_bwd, b_hh_bwd]

    w = np.zeros((2, 128, GCOLS), np.float32)
    bias = np.zeros(GCOLS, np.float32)
    bhn = np.zeros(1024, np.float32)
    for dl in range(4):
        l = dl % 2
        Wl = np.asarray(Wd[dl][l], np.float32)      # (3H, I)
        bil = np.asarray(bid[dl][l], np.float32)    # (3H,)
        bhl = np.asarray(bhd[dl][l], np.float32)
        sl = slice(dl * 256, (dl + 1) * 256)
        for k in range(2):
            isel = slice(k * 128, (k + 1) * 128)
            # R block: cols [0:1024)
            w[k, :, 0:1024][:, sl] = Wl[0:H, isel].T
            # Z block negated: cols [1024:2048)
            w[k, :, 1024:2048][:, sl] = -Wl[H : 2 * H, isel].T
            # N block: cols [2048:3072)
            w[k, :, 2048:3072][:, sl] = Wl[2 * H : 3 * H, isel].T
        bias[0:1024][sl] = bil[0:H] + bhl[0:H]
        bias[1024:2048][sl] = -(bil[H : 2 * H] + bhl[H : 2 * H])
        bias[2048:3072][sl] = bil[2 * H : 3 * H]
        bhn[sl] = bhl[2 * H : 3 * H]

    w_np = w.astype(BF16)
    bias_np = np.ascontiguousarray(np.broadcast_to(bias, (128, GCOLS)), np.float32)
    bhn_np = np.ascontiguousarray(np.broadcast_to(bhn, (128, 1024))).astype(BF16)
    return w_np, bias_np, bhn_np


def _build_nc():
    import concourse.bass as bass
    import concourse.mybir as mybir
    from concourse import bacc
    import concourse.tile as tile
    from concourse.alu_op_type import AluOpType

    AF = mybir.ActivationFunctionType
    f32 = mybir.dt.float32
    bf16 = mybir.dt.bfloat16

    nc = bacc.Bacc(
        "TRN2", target_bir_lowering=False, debug=False, num_devices=NCORES
    )
    x_in = nc.dram_tensor("x", [BPC, S, I], f32, kind="ExternalInput").ap()
    w_in = nc.dram_tensor("w", [2, 128, GCOLS], bf16, kind="ExternalInput").ap()
    bias_in = nc.dram_tensor("bias", [128, GCOLS], f32, kind="ExternalInput").ap()
    bhn_in = nc.dram_tensor("bhn", [128, 1024], bf16, kind="ExternalInput").ap()
    out_t = nc.dram_tensor("out", [BPC, 2 * S * L, H], f32, kind="ExternalOutput")

    OUT_B = 2 * S * L * H       # flat elems per batch row
    BWD_OFF = S * L * H         # flat offset of the bwd half within a batch row

    with tile.TileContext(nc) as tc:
        with (
            tc.tile_pool(name="const", bufs=1) as cpool,
            tc.tile_pool(name="xload", bufs=3) as xpool,
            tc.tile_pool(name="xt", bufs=4) as xtpool,
            tc.tile_pool(name="work", bufs=3) as wpool,
            tc.tile_pool(name="outp", bufs=4) as opool,
            tc.tile_pool(name="ps_r", bufs=1, space="PSUM") as prp,
            tc.tile_pool(name="ps_z", bufs=1, space="PSUM") as pzp,
            tc.tile_pool(name="ps_n", bufs=2, space="PSUM") as pnp,
        ):
            w0 = cpool.tile([128, GCOLS], bf16, name="w0")
            w1 = cpool.tile([128, GCOLS], bf16, name="w1")
            bias_sb = cpool.tile([128, GCOLS], f32, name="bias_sb")
            bhn_sb = cpool.tile([128, 1024], bf16, name="bhn_sb")
            nc.sync.dma_start(out=w0[:], in_=w_in[0])
            nc.sync.dma_start(out=w1[:], in_=w_in[1])
            nc.sync.dma_start(out=bias_sb[:], in_=bias_in)
            nc.sync.dma_start(out=bhn_sb[:], in_=bhn_in)
            wk = [w0, w1]

            for it4 in range(NTILES // 4):
                b = (it4 * 4) // SB_PER_B
                sb4 = (it4 * 4) % SB_PER_B
                xin4 = xpool.tile([128, 4 * I], bf16, name="xin4")
                src = x_in[b, sb4 * NT : (sb4 + 4) * NT, :].rearrange(
                    "(j p) i -> p j i", p=128
                )
                # SWDGE cast DMA: f32 DRAM -> bf16 SBUF
                nc.gpsimd.dma_start(out=xin4[:], in_=src)

                for j in range(4):
                    t0 = (sb4 + j) * NT
                    xT = xtpool.tile([128, 2 * NT], bf16, name="xT")
                    for k in range(2):
                        nc.sync.dma_start_transpose(
                            out=xT[:, k * NT : (k + 1) * NT],
                            in_=xin4[:, j * I + k * 128 : j * I + (k + 1) * 128],
                        )

                    ps_n = pnp.tile([128, 1024], f32, name="ps_n")
                    ps_r = prp.tile([128, 1024], f32, name="ps_r")
                    ps_z = pzp.tile([128, 1024], f32, name="ps_z")
                    ps_gt = [ps_r, ps_z, ps_n]
                    # Gate-column 512-blocks alternate fwd/bwd:
                    #   rz: [r-fwd, r-bwd, z-fwd, z-bwd], n: [n-fwd, n-bwd].
                    # Bwd blocks use the column-REVERSED stationary xT so psum
                    # partition p holds token t0+127-p; the elementwise chain
                    # is pointwise so this stays consistent, and the bwd store
                    # becomes an ascending-stride DMA.
                    # column-reversed copy of xT (per k-chunk) for bwd blocks;
                    # matmul weight APs reject negative strides, so materialize
                    # via a DVE copy (step -1 input is a supported fast path).
                    xTr = xtpool.tile([128, 2 * NT], bf16, name="xTr")
                    for k in range(2):
                        rev_view = bass.AP(
                            xT.tensor,
                            xT.offset + (k + 1) * NT - 1,
                            [list(xT.ap[0]), [-1, NT]],
                        )
                        nc.vector.tensor_copy(xTr[:, k * NT : (k + 1) * NT], rev_view)
                    for k in range(2):
                        xk = xT[:, k * NT : (k + 1) * NT]
                        xkr = xTr[:, k * NT : (k + 1) * NT]
                        for rev in (0, 1):
                            lhsT = xkr if rev else xk
                            for gt in range(3):  # r, z, n blocks
                                col = gt * 1024 + rev * 512
                                dst = ps_gt[gt][:, rev * 512 : (rev + 1) * 512]
                                nc.tensor.matmul(
                                    dst,
                                    lhsT,
                                    wk[k][:, col : col + 512],
                                    start=(k == 0),
                                    stop=(k == 1),
                                )

                    rz_pre = wpool.tile([128, 2048], bf16, name="rz_pre")
                    nc.vector.tensor_tensor(
                        rz_pre[:, 0:1024], ps_r[:], bias_sb[:, 0:1024], AluOpType.add
                    )
                    nc.vector.tensor_tensor(
                        rz_pre[:, 1024:2048],
                        ps_z[:],
                        bias_sb[:, 1024:2048],
                        AluOpType.add,
                    )
                    nb_sb = wpool.tile([128, 1024], bf16, name="nb_sb")
                    nc.vector.tensor_tensor(
                        nb_sb[:], ps_n[:], bias_sb[:, 2048:GCOLS], AluOpType.add
                    )
                    rz_act = wpool.tile([128, 2048], bf16, name="rz_act")
                    nc.scalar.activation(rz_act[:], rz_pre[:], AF.Sigmoid)
                    tmul = wpool.tile([128, 1024], bf16, name="tmul")
                    nc.gpsimd.tensor_tensor(
                        tmul[:, 0:896], rz_act[:, 0:896], bhn_sb[:, 0:896],
                        AluOpType.mult,
                    )
                    nc.vector.tensor_tensor(
                        tmul[:, 896:1024],
                        rz_act[:, 896:1024],
                        bhn_sb[:, 896:1024],
                        AluOpType.mult,
                    )
                    pre_n = wpool.tile([128, 1024], bf16, name="pre_n")
                    nc.vector.tensor_tensor(
                        pre_n[:], nb_sb[:], tmul[:], AluOpType.add
                    )
                    n_sb = wpool.tile([128, 1024], bf16, name="n_sb")
                    nc.scalar.activation(n_sb[:], pre_n[:], AF.Tanh)
                    out_sb = opool.tile([128, 1024], f32, name="out_sb")
                    nc.gpsimd.tensor_tensor(
                        out_sb[:], rz_act[:, 1024:2048], n_sb[:], AluOpType.mult
                    )

                    base = b * OUT_B
                    fwd = bass.AP(out_t, base + t0 * 512, [[512, 128], [1, 512]])
                    nc.sync.dma_start(out=fwd, in_=out_sb[:, 0:512])
                    # bwd partitions hold tokens reversed (p <-> t0+127-p), so
                    # dest rows q = S - t0 - 127 + p ascend with p.
                    bbase = base + BWD_OFF
                    if t0 == 0:
                        # p=0..126 -> q=3969..4095 ; p=127 (token 0) -> q=0
                        rest = bass.AP(
                            out_t, bbase + (S - 127) * 512, [[512, 127], [1, 512]]
                        )
                        nc.sync.dma_start(out=rest, in_=out_sb[0:127, 512:1024])
                        one = bass.AP(out_t, bbase, [[512, 1], [1, 512]])
                        nc.sync.dma_start(out=one, in_=out_sb[127:128, 512:1024])
                    else:
                        bwd = bass.AP(
                            out_t,
                            bbase + (S - t0 - 127) * 512,
                            [[512, 128], [1, 512]],
                        )
                        nc.sync.dma_start(out=bwd, in_=out_sb[:, 512:1024])

    nc.compile()
    return nc


def _get_nc():
    if "nc" not in _CACHE:
        _CACHE["nc"] = _build_nc()
    return _CACHE["nc"]


def kernel(
    input,
    W_ih_fwd,
    W_hh_fwd,
    b_ih_fwd,
    b_hh_fwd,
    W_ih_bwd,
    W_hh_bwd,
    b_ih_bwd,
    b_hh_bwd,
    _trace=False,
):
    from concourse.bass_utils import run_bass_kernel_spmd

    x = np.asarray(input, np.float32)
    w_np, bias_np, bhn_np = _prep_weights(
        np.asarray(W_ih_fwd, np.float32),
        np.asarray(b_ih_fwd, np.float32),
        np.asarray(b_hh_fwd, np.float32),
        np.asarray(W_ih_bwd, np.float32),
        np.asarray(b_ih_bwd, np.float32),
        np.asarray(b_hh_bwd, np.float32),
    )

    nc = _get_nc()
    in_maps = []
    for c in range(NCORES):
        in_maps.append(
            {
                "x": np.ascontiguousarray(x[c * BPC : (c + 1) * BPC]),
                "w": w_np,
                "bias": bias_np,
                "bhn": bhn_np,
            }
        )
    res = run_bass_kernel_spmd(
        nc, in_maps, core_ids=list(range(NCORES)), trace=_trace
    )
    out = np.concatenate([r["out"] for r in res.results], axis=0)
    if _trace:
        _CACHE["last_results"] = res
    return out
